# revision 9
# baseline (speedup 1.0000x reference)
"""MinGRU Trainium2 kernel.

Reference computation (B=8, T=4096, D=H=1024):
    k        = x @ W_z.T + b_z
    z        = sigmoid(k);  coeff = 1 - z
    tilde_h  = g(x @ W_h.T + b_h)   where g(u) = max(u + 0.5, sigmoid(u))
    h_t      = coeff_t * h_{t-1} + z_t * tilde_h_t,  h_init = g(h_0)
    output   = [g(h_0), h_1 .. h_T]  per batch  -> [B, T+1, H]

The reference evaluates the scan in log space purely for numerical
stability; the direct-space recurrence is a convex combination at every
step (contraction), so fp32 direct evaluation matches to ~1e-6.

Sharding: data-parallel over batch. Core b computes batch b; there is no
cross-core communication. On-device work per core:
    two [4096,1024]x[1024,1024] matmuls in float32r (full-rate, ~1.6e-4
    relative), sigmoids on ScalarE, coeff/v on GpSimd, g() on VectorE,
    and the time recurrence as hardware TensorTensorScan instructions.
Layout on device is [H partitions, T free] (the scan runs along the free
axis); the host pre-transposes x and W and re-transposes the output.

Per time-chunk the matmuls run as a W_z sweep over all 8 H-blocks, then
a W_h sweep — so the first matmul of the kernel only gates on x-chunk0
plus W_z (6 MB of DMA), while W_h streams in behind it.
"""

import numpy as np

B, T, D, H = 8, 4096, 1024, 1024
NCORES = 8
PB = 128          # partition block
KB = D // PB      # contraction blocks (8)
HB = H // PB      # output-row blocks (8)
TCHUNK = 512      # moving free-dim per matmul / scan chunk
NT = T // TCHUNK  # 8 time chunks

_cache = {}


def _build_bass():
    import concourse.tile as tile
    import concourse.mybir as mybir
    from concourse import bacc

    f32 = mybir.dt.float32
    f32r = mybir.dt.float32r
    ACT = mybir.ActivationFunctionType
    OP = mybir.AluOpType

    nc = bacc.Bacc("TRN2", target_bir_lowering=False, debug=False,
                   num_devices=NCORES)

    xT = nc.dram_tensor("xT", [D, T], f32r, kind="ExternalInput")
    wzT = nc.dram_tensor("wzT", [D, H], f32r, kind="ExternalInput")
    whT = nc.dram_tensor("whT", [D, H], f32r, kind="ExternalInput")
    # packed per-partition constants: columns = [b_z | b_h+0.5 | b_h | g(h0)]
    # each group has HB columns, one per 128-row H block
    consts = nc.dram_tensor("consts", [PB, 4 * HB], f32, kind="ExternalInput")
    hT = nc.dram_tensor("hT", [H, T], f32, kind="ExternalOutput")

    with tile.TileContext(nc) as tc:
        with (
            tc.tile_pool(name="wpool", bufs=1) as wpool,
            tc.tile_pool(name="cpool", bufs=1) as cpool,
            tc.tile_pool(name="xpool", bufs=2) as xpool,
            tc.tile_pool(name="zpool", bufs=10) as zpool,
            tc.tile_pool(name="spool", bufs=3) as spool,
            tc.tile_pool(name="hpool", bufs=2) as hpool,
            tc.tile_pool(name="psum", bufs=8, space="PSUM") as psum,
        ):
            cb = cpool.tile([PB, 4 * HB], f32, tag="consts")
            nc.sync.dma_start(cb[:], consts[:])

            def bias_bz(h):
                return cb[:, h:h + 1]

            def bias_bh05(h):
                return cb[:, HB + h:HB + h + 1]

            def bias_bh(h):
                return cb[:, 2 * HB + h:2 * HB + h + 1]

            def init_g0(h):
                return cb[:, 3 * HB + h:3 * HB + h + 1]

            # chunk-0 x and W_z interleaved per k-block: chunk 0 runs its
            # matmuls k-outer so the PE consumes each (x0[k], wz[k]) pair
            # as soon as the DMA delivers it; W_h queues behind them
            xT_blk = xT.ap().rearrange("(k p) t -> p k t", p=PB)      # [PB, KB, T]
            hT_blk = hT.ap().rearrange("(hb p) t -> p hb t", p=PB)    # [PB, HB, T]
            x_tiles = [None] * NT
            x_tiles[0] = xpool.tile([PB, KB, TCHUNK], f32r, tag="x", name="x0")
            wz_sb = wpool.tile([PB, KB, H], f32r, tag="wz")
            for k in range(KB):
                nc.sync.dma_start(
                    x_tiles[0][:, k, :], xT[k * PB:(k + 1) * PB, 0:TCHUNK])
                nc.sync.dma_start(wz_sb[:, k, :], wzT[k * PB:(k + 1) * PB, :])
            wh_sb = wpool.tile([PB, KB, H], f32r, tag="wh")
            for k in range(KB):
                nc.sync.dma_start(wh_sb[:, k, :], whT[k * PB:(k + 1) * PB, :])

            h_prev = None
            for t in range(NT):
                ts0 = t * TCHUNK
                x_sb = x_tiles[t]
                k_outer = (t == 0)

                # ---- W_z sweep: z and coeff for all H blocks ----
                zs, cs = [None] * HB, [None] * HB
                pks = [None] * HB
                for k, h in (((k_, h_) for k_ in range(KB) for h_ in range(HB))
                             if k_outer else
                             ((k_, h_) for h_ in range(HB) for k_ in range(KB))):
                    hs = slice(h * PB, (h + 1) * PB)
                    if pks[h] is None:
                        pks[h] = psum.tile([PB, TCHUNK], f32, tag="ps",
                                           name=f"pk_{t}_{h}")
                    nc.tensor.matmul(pks[h][:], wz_sb[:, k, hs], x_sb[:, k, :],
                                     start=(k == 0), stop=(k == KB - 1))
                    if k == KB - 1:
                        z = zpool.tile([PB, TCHUNK], f32, tag="z")
                        nc.scalar.activation(z[:], pks[h][:], ACT.Sigmoid,
                                             bias=bias_bz(h), scale=1.0)
                        c = zpool.tile([PB, TCHUNK], f32, tag="c")
                        nc.gpsimd.tensor_scalar(c[:], z[:], -1.0, 1.0,
                                                op0=OP.mult, op1=OP.add)
                        zs[h], cs[h] = z, c

                if t + 1 < NT:  # prefetch next x chunk (queues behind W_h)
                    x_tiles[t + 1] = xpool.tile([PB, KB, TCHUNK], f32r, tag="x",
                                                name=f"x{t + 1}")
                    nc.sync.dma_start(
                        x_tiles[t + 1][:],
                        xT_blk[:, :, ts0 + TCHUNK:ts0 + 2 * TCHUNK])

                # ---- W_h sweep: tilde, v, scan, store ----
                hall = hpool.tile([PB, HB, TCHUNK], f32, tag="hall",
                                  name=f"hall{t}")
                pps = [None] * HB
                for k, h in (((k_, h_) for k_ in range(KB) for h_ in range(HB))
                             if k_outer else
                             ((k_, h_) for h_ in range(HB) for k_ in range(KB))):
                    hs = slice(h * PB, (h + 1) * PB)
                    if pps[h] is None:
                        pps[h] = psum.tile([PB, TCHUNK], f32, tag="ps",
                                           name=f"pp_{t}_{h}")
                    nc.tensor.matmul(pps[h][:], wh_sb[:, k, hs], x_sb[:, k, :],
                                     start=(k == 0), stop=(k == KB - 1))
                    if k != KB - 1:
                        continue
                    pp = pps[h]
                    sp = spool.tile([PB, TCHUNK], f32, tag="sp")
                    nc.scalar.activation(sp[:], pp[:], ACT.Sigmoid,
                                         bias=bias_bh(h), scale=1.0)
                    # tilde = max(pre + b_h + 0.5, sigmoid(pre + b_h))
                    tilde = spool.tile([PB, TCHUNK], f32, tag="tilde")
                    nc.vector.scalar_tensor_tensor(
                        tilde[:], pp[:], bias_bh05(h), sp[:],
                        op0=OP.add, op1=OP.max)
                    v = spool.tile([PB, TCHUNK], f32, tag="v")
                    nc.vector.tensor_mul(v[:], zs[h][:], tilde[:])

                    init = (init_g0(h) if t == 0
                            else h_prev[:, h, TCHUNK - 1:TCHUNK])
                    nc.vector.tensor_tensor_scan(
                        hall[:, h, :], cs[h][:], v[:], init,
                        op0=OP.mult, op1=OP.add)
                    if h == HB - 1:
                        h_prev = hall
                        nc.sync.dma_start(
                            hT_blk[:, :, ts0:ts0 + TCHUNK], hall[:])

    nc.compile()
    return nc


def _get_nc():
    if "nc" not in _cache:
        _cache["nc"] = _build_bass()
    return _cache["nc"]


def _prep_inputs(x, h_0, W_z, b_z, W_h, b_h):
    x = np.asarray(x, dtype=np.float32)
    h_0 = np.asarray(h_0, dtype=np.float32)
    W_z = np.asarray(W_z, dtype=np.float32)
    b_z = np.asarray(b_z, dtype=np.float32)
    W_h = np.asarray(W_h, dtype=np.float32)
    b_h = np.asarray(b_h, dtype=np.float32)

    wzT = np.ascontiguousarray(W_z.T)              # [D, H]
    whT = np.ascontiguousarray(W_h.T)

    h0f = h_0.reshape(B, H)
    g0 = np.where(h0f >= 0.0, h0f + np.float32(0.5),
                  1.0 / (1.0 + np.exp(-h0f))).astype(np.float32)  # [B, H]

    def blocked(vec):  # [H] -> [PB, HB] column per block
        return np.ascontiguousarray(vec.reshape(HB, PB).T)

    in_maps = []
    for b in range(B):
        consts = np.concatenate(
            [blocked(b_z), blocked(b_h + np.float32(0.5)), blocked(b_h),
             blocked(g0[b])], axis=1).astype(np.float32)  # [PB, 4*HB]
        in_maps.append({
            "xT": np.ascontiguousarray(x[b].T),    # [D, T]
            "wzT": wzT, "whT": whT,
            "consts": consts,
        })
    return in_maps, g0


def kernel(x, h_0, W_z, b_z, W_h, b_h):
    from concourse.bass_utils import run_bass_kernel_spmd

    in_maps, g0 = _prep_inputs(x, h_0, W_z, b_z, W_h, b_h)
    nc = _get_nc()
    res = run_bass_kernel_spmd(nc, in_maps, core_ids=list(range(NCORES)))
    _cache["last_results"] = res

    out = np.empty((B, T + 1, H), dtype=np.float32)
    for b in range(B):
        out[b, 0, :] = g0[b]
        out[b, 1:, :] = res.results[b]["hT"].T
    return out


# revision 10
# speedup vs baseline: 1.2222x; 1.2222x over previous
"""MinGRU Trainium2 kernel.

Reference computation (B=8, T=4096, D=H=1024):
    k        = x @ W_z.T + b_z
    z        = sigmoid(k);  coeff = 1 - z
    tilde_h  = g(x @ W_h.T + b_h)   where g(u) = max(u + 0.5, sigmoid(u))
    h_t      = coeff_t * h_{t-1} + z_t * tilde_h_t,  h_init = g(h_0)
    output   = [g(h_0), h_1 .. h_T]  per batch  -> [B, T+1, H]

The reference evaluates the scan in log space purely for numerical
stability; the direct-space recurrence is a convex combination at every
step (contraction), so fp32 direct evaluation matches to ~1e-6.

Sharding: data-parallel over batch. Core b computes batch b; there is no
cross-core communication. On-device work per core:
    two [4096,1024]x[1024,1024] matmuls in float32r (full-rate, ~1.6e-4
    relative), sigmoids on ScalarE, coeff/v on GpSimd, g() on VectorE,
    and the time recurrence as hardware TensorTensorScan instructions.
Layout on device is [H partitions, T free] (the scan runs along the free
axis); the host pre-transposes x and W and re-transposes the output.

Per time-chunk the matmuls run as a W_z sweep over all 8 H-blocks, then
a W_h sweep — so the first matmul of the kernel only gates on x-chunk0
plus W_z (6 MB of DMA), while W_h streams in behind it.
"""

import numpy as np

B, T, D, H = 8, 4096, 1024, 1024
NCORES = 8
PB = 128          # partition block
KB = D // PB      # contraction blocks (8)
HB = H // PB      # output-row blocks (8)
TCHUNK = 512      # moving free-dim per matmul / scan chunk
NT = T // TCHUNK  # 8 time chunks

_cache = {}


def _build_bass():
    import concourse.tile as tile
    import concourse.mybir as mybir
    from concourse import bacc

    f32 = mybir.dt.float32
    f32r = mybir.dt.float32r
    ACT = mybir.ActivationFunctionType
    OP = mybir.AluOpType

    nc = bacc.Bacc("TRN2", target_bir_lowering=False, debug=False,
                   num_devices=NCORES)

    xT = nc.dram_tensor("xT", [D, T], f32r, kind="ExternalInput")
    wzT = nc.dram_tensor("wzT", [D, H], f32r, kind="ExternalInput")
    whT = nc.dram_tensor("whT", [D, H], f32r, kind="ExternalInput")
    # packed per-partition constants: columns = [b_z | b_h+0.5 | b_h | g(h0)]
    # each group has HB columns, one per 128-row H block
    consts = nc.dram_tensor("consts", [PB, 4 * HB], f32, kind="ExternalInput")
    hT = nc.dram_tensor("hT", [H, T], f32, kind="ExternalOutput")

    with tile.TileContext(nc) as tc:
        with (
            tc.tile_pool(name="wpool", bufs=1) as wpool,
            tc.tile_pool(name="cpool", bufs=1) as cpool,
            tc.tile_pool(name="xpool", bufs=2) as xpool,
            tc.tile_pool(name="zpool", bufs=10) as zpool,
            tc.tile_pool(name="spool", bufs=3) as spool,
            tc.tile_pool(name="hpool", bufs=2) as hpool,
            tc.tile_pool(name="psum", bufs=8, space="PSUM") as psum,
        ):
            cb = cpool.tile([PB, 4 * HB], f32, tag="consts")
            nc.sync.dma_start(cb[:], consts[:])

            def bias_bz(h):
                return cb[:, h:h + 1]

            def bias_bh05(h):
                return cb[:, HB + h:HB + h + 1]

            def bias_bh(h):
                return cb[:, 2 * HB + h:2 * HB + h + 1]

            def init_g0(h):
                return cb[:, 3 * HB + h:3 * HB + h + 1]

            # chunk-0 x and W_z interleaved per k-block: chunk 0 runs its
            # matmuls k-outer so the PE consumes each (x0[k], wz[k]) pair
            # as soon as the DMA delivers it; W_h queues behind them
            xT_blk = xT.ap().rearrange("(k p) t -> p k t", p=PB)      # [PB, KB, T]
            hT_blk = hT.ap().rearrange("(hb p) t -> p hb t", p=PB)    # [PB, HB, T]
            x_tiles = [None] * NT
            x_tiles[0] = xpool.tile([PB, KB, TCHUNK], f32r, tag="x", name="x0")
            wz_sb = wpool.tile([PB, KB, H], f32r, tag="wz")
            for k in range(KB):
                nc.sync.dma_start(
                    x_tiles[0][:, k, :], xT[k * PB:(k + 1) * PB, 0:TCHUNK])
                nc.sync.dma_start(wz_sb[:, k, :], wzT[k * PB:(k + 1) * PB, :])
            wh_sb = wpool.tile([PB, KB, H], f32r, tag="wh")
            for k in range(KB):
                nc.sync.dma_start(wh_sb[:, k, :], whT[k * PB:(k + 1) * PB, :])

            h_prev = [None] * HB
            for t in range(NT):
                ts0 = t * TCHUNK
                x_sb = x_tiles[t]
                k_outer = (t == 0)

                # ---- W_z sweep: z and coeff for all H blocks ----
                zs, cs = [None] * HB, [None] * HB
                pks = [None] * HB
                for k, h in (((k_, h_) for k_ in range(KB) for h_ in range(HB))
                             if k_outer else
                             ((k_, h_) for h_ in range(HB) for k_ in range(KB))):
                    hs = slice(h * PB, (h + 1) * PB)
                    if pks[h] is None:
                        pks[h] = psum.tile([PB, TCHUNK], f32, tag="ps",
                                           name=f"pk_{t}_{h}")
                    nc.tensor.matmul(pks[h][:], wz_sb[:, k, hs], x_sb[:, k, :],
                                     start=(k == 0), stop=(k == KB - 1))
                    if k == KB - 1:
                        z = zpool.tile([PB, TCHUNK], f32, tag="z")
                        nc.scalar.activation(z[:], pks[h][:], ACT.Sigmoid,
                                             bias=bias_bz(h), scale=1.0)
                        c = zpool.tile([PB, TCHUNK], f32, tag="c")
                        nc.gpsimd.tensor_scalar(c[:], z[:], -1.0, 1.0,
                                                op0=OP.mult, op1=OP.add)
                        zs[h], cs[h] = z, c

                if t + 1 < NT:  # prefetch next x chunk (queues behind W_h)
                    x_tiles[t + 1] = xpool.tile([PB, KB, TCHUNK], f32r, tag="x",
                                                name=f"x{t + 1}")
                    for k in range(KB):
                        nc.sync.dma_start(
                            x_tiles[t + 1][:, k, :],
                            xT[k * PB:(k + 1) * PB, ts0 + TCHUNK:ts0 + 2 * TCHUNK])

                # ---- W_h sweep: tilde, v, scan, store ----
                pps = [None] * HB
                for k, h in (((k_, h_) for k_ in range(KB) for h_ in range(HB))
                             if k_outer else
                             ((k_, h_) for h_ in range(HB) for k_ in range(KB))):
                    hs = slice(h * PB, (h + 1) * PB)
                    if pps[h] is None:
                        pps[h] = psum.tile([PB, TCHUNK], f32, tag="ps",
                                           name=f"pp_{t}_{h}")
                    nc.tensor.matmul(pps[h][:], wh_sb[:, k, hs], x_sb[:, k, :],
                                     start=(k == 0), stop=(k == KB - 1))
                    if k != KB - 1:
                        continue
                    pp = pps[h]
                    sp = spool.tile([PB, TCHUNK], f32, tag="sp")
                    nc.scalar.activation(sp[:], pp[:], ACT.Sigmoid,
                                         bias=bias_bh(h), scale=1.0)
                    # tilde = max(pre + b_h + 0.5, sigmoid(pre + b_h))
                    tilde = spool.tile([PB, TCHUNK], f32, tag="tilde")
                    nc.vector.scalar_tensor_tensor(
                        tilde[:], pp[:], bias_bh05(h), sp[:],
                        op0=OP.add, op1=OP.max)
                    v = spool.tile([PB, TCHUNK], f32, tag="v")
                    nc.vector.tensor_mul(v[:], zs[h][:], tilde[:])

                    hout = hpool.tile([PB, TCHUNK], f32, tag=f"h{h}",
                                      name=f"h_{t}_{h}")
                    init = (init_g0(h) if t == 0
                            else h_prev[h][:, TCHUNK - 1:TCHUNK])
                    nc.vector.tensor_tensor_scan(
                        hout[:], cs[h][:], v[:], init,
                        op0=OP.mult, op1=OP.add)
                    h_prev[h] = hout
                    nc.sync.dma_start(hT[hs, ts0:ts0 + TCHUNK], hout[:])

    nc.compile()
    return nc


def _get_nc():
    if "nc" not in _cache:
        _cache["nc"] = _build_bass()
    return _cache["nc"]


def _prep_inputs(x, h_0, W_z, b_z, W_h, b_h):
    x = np.asarray(x, dtype=np.float32)
    h_0 = np.asarray(h_0, dtype=np.float32)
    W_z = np.asarray(W_z, dtype=np.float32)
    b_z = np.asarray(b_z, dtype=np.float32)
    W_h = np.asarray(W_h, dtype=np.float32)
    b_h = np.asarray(b_h, dtype=np.float32)

    wzT = np.ascontiguousarray(W_z.T)              # [D, H]
    whT = np.ascontiguousarray(W_h.T)

    h0f = h_0.reshape(B, H)
    g0 = np.where(h0f >= 0.0, h0f + np.float32(0.5),
                  1.0 / (1.0 + np.exp(-h0f))).astype(np.float32)  # [B, H]

    def blocked(vec):  # [H] -> [PB, HB] column per block
        return np.ascontiguousarray(vec.reshape(HB, PB).T)

    in_maps = []
    for b in range(B):
        consts = np.concatenate(
            [blocked(b_z), blocked(b_h + np.float32(0.5)), blocked(b_h),
             blocked(g0[b])], axis=1).astype(np.float32)  # [PB, 4*HB]
        in_maps.append({
            "xT": np.ascontiguousarray(x[b].T),    # [D, T]
            "wzT": wzT, "whT": whT,
            "consts": consts,
        })
    return in_maps, g0


def kernel(x, h_0, W_z, b_z, W_h, b_h):
    from concourse.bass_utils import run_bass_kernel_spmd

    in_maps, g0 = _prep_inputs(x, h_0, W_z, b_z, W_h, b_h)
    nc = _get_nc()
    res = run_bass_kernel_spmd(nc, in_maps, core_ids=list(range(NCORES)))
    _cache["last_results"] = res

    out = np.empty((B, T + 1, H), dtype=np.float32)
    for b in range(B):
        out[b, 0, :] = g0[b]
        out[b, 1:, :] = res.results[b]["hT"].T
    return out
